# revision 1
# baseline (speedup 1.0000x reference)
"""BiLSTM-CRF loss kernel for Trainium2 (8 NeuronCores, SPMD data-parallel).

Full inputs -> full scalar output. Sharding: batch 32 -> 4 rows/core x 8 cores.

Per-core pipeline (v3):
  gather embeddings (indirect DMA) -> PE-transpose -> input projections Gx
  (fp8 weights) -> both LSTM directions step-interleaved (two independent
  dependency chains keep all engines busy) -> linear projection + exp
  emissions -> linear-space CRF forward scan -> per-batch partials.

LSTM step: the gates accumulation group starts with an identity-matmul that
injects the precomputed Gx block through the PE (no vector-engine add on the
recurrent critical path), followed by 16 fp8 Whh tile matmuls (fast weight
load).  All four gates go through ONE sigmoid straight from PSUM (the g-gate
rows are pre-scaled by 2 on the host; tanh(x) = 2 sigmoid(2x) - 1 is applied
as a fused tensor_scalar op).

CRF: beta_t = EM_t * (ET'^T @ beta_{t-1}) in linear space with ET' =
exp(trans - log K) stationary on the PE; logZ = log(sum beta_T * exp(end)) +
(T-1) log K.  State stays within e^+-4 (validated numerically).  The batch is
split into two independent scan chains to halve the serial latency.
"""

import numpy as np
import ml_dtypes

VOCAB, EMB, HID, K, B, T = 30000, 256, 512, 9, 32, 512
H = HID // 2          # 256 per-direction hidden
NCORES = 8
BC = B // NCORES      # 4 batch rows per core
LOG_K = float(np.log(K))
# m-chunk order in the gates psum tile: [i0 i1 f0 f1 o0 o1 g0 g1]
MORDER = [0, 1, 2, 3, 6, 7, 4, 5]

F8 = ml_dtypes.float8_e4m3
BF16 = ml_dtypes.bfloat16

_CACHE = {}


def _build_module(t_steps=T, repeat=1, gx_inject=True, one_sig=True,
                  crf_chains=2, stop_after='full', lstm_reps=1, crf_reps=1):
    import concourse.bacc as bacc
    import concourse.tile as tile
    import concourse.mybir as mybir
    from concourse import bass
    from concourse.masks import make_identity

    dt = mybir.dt
    AF = mybir.ActivationFunctionType
    NT = t_steps * BC  # flattened (t, b) columns per core

    nc = bacc.Bacc("TRN2", target_bir_lowering=False, debug=False,
                   num_devices=NCORES)

    d_emb = nc.dram_tensor("embq", [VOCAB, EMB], dt.bfloat16, kind="ExternalInput").ap()
    d_tidx = nc.dram_tensor("tidx", [128, NT // 128], dt.int32, kind="ExternalInput").ap()
    d_wih = nc.dram_tensor("wih", [128, 2, 2, 8, 128], dt.float8e4, kind="ExternalInput").ap()
    d_whh = nc.dram_tensor("whh", [128, 2, 2, 8, 128], dt.float8e4, kind="ExternalInput").ap()
    d_gbias = nc.dram_tensor("gbias", [128, 2, 8], dt.float32, kind="ExternalInput").ap()
    d_wlin = nc.dram_tensor("wlin", [128, 4, K], dt.float8e4, kind="ExternalInput").ap()
    d_blin = nc.dram_tensor("blin", [K, 1], dt.float32, kind="ExternalInput").ap()
    d_et = nc.dram_tensor("et", [K, K], dt.float32, kind="ExternalInput").ap()
    d_estart = nc.dram_tensor("estart", [K, 1], dt.float32, kind="ExternalInput").ap()
    d_eend = nc.dram_tensor("eend", [K, 1], dt.float32, kind="ExternalInput").ap()
    d_oht = nc.dram_tensor("oht", [K, NT], dt.float32, kind="ExternalInput").ap()
    d_h0 = nc.dram_tensor("h0q", [128, 2, 2, BC], dt.bfloat16, kind="ExternalInput").ap()
    d_c0 = nc.dram_tensor("c0i", [128, 2, 2, BC], dt.float32, kind="ExternalInput").ap()
    d_res = nc.dram_tensor("res", [1, 2 * BC], dt.float32, kind="ExternalOutput").ap()

    with tile.TileContext(nc) as tc:
        from contextlib import ExitStack
        with ExitStack() as ctx:
            pconst = ctx.enter_context(tc.tile_pool(name="pconst", bufs=1))

            # ---- persistent SBUF tensors ----
            sb_wih = pconst.tile([128, 2, 2, 8, 128], dt.float8e4)
            sb_whh = pconst.tile([128, 2, 2, 8, 128], dt.float8e4)
            sb_gbias = pconst.tile([128, 2, 8], dt.float32)
            sb_wlin = pconst.tile([128, 4, K], dt.float8e4)
            sb_blin = pconst.tile([K, 1], dt.float32)
            sb_et = pconst.tile([K, K], dt.float32)
            sb_estart = pconst.tile([K, 1], dt.float32)
            sb_eend = pconst.tile([K, 1], dt.float32)
            sb_ones9 = pconst.tile([K, 1], dt.float32)
            sb_oht = pconst.tile([K, NT], dt.float32)
            sb_tidx = pconst.tile([128, NT // 128], dt.int32)
            sb_h0 = pconst.tile([128, 2, 2, BC], dt.bfloat16)
            sb_c = pconst.tile([128, 2, 2, BC], dt.float32)   # running c state
            sb_ident = pconst.tile([128, 128], dt.bfloat16)   # for PE transpose
            sb_ident8 = pconst.tile([128, 128], dt.float8e4)  # for Gx injection
            sb_xT = pconst.tile([128, 2, NT], dt.bfloat16)
            sb_gx = pconst.tile([128, 2, 8, NT], dt.bfloat16)
            sb_hsT = pconst.tile([128, 2, 2, NT], dt.bfloat16)  # [p, dir, khalf, col]
            sb_em = pconst.tile([K, NT], dt.float32)
            sb_sel = pconst.tile([K, NT], dt.float32)
            sb_emsum = pconst.tile([K, BC], dt.float32)
            sb_res = pconst.tile([1, 2 * BC], dt.float32)
            beta = pconst.tile([K, BC], dt.float32)

            eng = nc.sync  # DMA queue engine
            eng.dma_start(out=sb_wih[:], in_=d_wih)
            eng.dma_start(out=sb_whh[:], in_=d_whh)
            eng.dma_start(out=sb_gbias[:], in_=d_gbias)
            eng.dma_start(out=sb_wlin[:], in_=d_wlin)
            eng.dma_start(out=sb_blin[:], in_=d_blin)
            eng.dma_start(out=sb_et[:], in_=d_et)
            eng.dma_start(out=sb_estart[:], in_=d_estart)
            eng.dma_start(out=sb_eend[:], in_=d_eend)
            eng.dma_start(out=sb_oht[:], in_=d_oht)
            eng.dma_start(out=sb_tidx[:], in_=d_tidx)
            eng.dma_start(out=sb_h0[:], in_=d_h0)
            eng.dma_start(out=sb_c[:], in_=d_c0)
            nc.vector.memset(sb_ones9[:], 1.0)
            if stop_after != 'full':
                nc.vector.memset(sb_res[:], 0.0)
            make_identity(nc, sb_ident[:])
            make_identity(nc, sb_ident8[:])

            NCH = min(512, NT)

            for _rep in range(repeat):
                # ---- phase A: gather + transpose ----
                with tc.tile_pool(name="pgather", bufs=4) as pg, \
                     tc.tile_pool(name="pg_ps", bufs=4, space="PSUM") as pgp:
                    for i in range(NT // 128):
                        xg = pg.tile([128, EMB], dt.bfloat16, tag="xg")
                        nc.gpsimd.indirect_dma_start(
                            out=xg[:],
                            out_offset=None,
                            in_=d_emb,
                            in_offset=bass.IndirectOffsetOnAxis(
                                ap=sb_tidx[:, i:i + 1], axis=0),
                        )
                        for k in range(2):
                            pst = pgp.tile([128, 128], dt.bfloat16, tag="pst")
                            nc.tensor.transpose(
                                out=pst[:], in_=xg[:, 128 * k:128 * (k + 1)],
                                identity=sb_ident[:])
                            nc.scalar.copy(
                                out=sb_xT[:, k, 128 * i:128 * (i + 1)],
                                in_=pst[:])

                if stop_after == 'gather':
                    continue
                # ---- phase B: input projections for both directions ----
                with tc.tile_pool(name="pproj", bufs=4, space="PSUM") as ppp:
                    for d in range(2):
                        for m in range(8):
                            for n0 in range(0, NT, NCH):
                                psp = ppp.tile([128, NCH], dt.float32, tag="psp")
                                for k in range(2):
                                    nc.tensor.matmul(
                                        psp[:], lhsT=sb_wih[:, d, k, m, :],
                                        rhs=sb_xT[:, k, n0:n0 + NCH],
                                        start=(k == 0), stop=(k == 1))
                                nc.scalar.activation(
                                    sb_gx[:, d, m, n0:n0 + NCH], psp[:],
                                    AF.Identity, bias=sb_gbias[:, d, m:m + 1])

                if stop_after == 'proj':
                    continue
                # ---- phase C: both LSTM recurrences, step-interleaved ----
                for _lr in range(lstm_reps):
                  with tc.tile_pool(name="plstm", bufs=4) as pl, \
                     tc.tile_pool(name="plstm_ps", bufs=4, space="PSUM") as plp:
                    for s in range(t_steps):
                        for d in range(2):
                            t = s if d == 0 else t_steps - 1 - s
                            if s == 0:
                                rhs_prev = sb_h0[:, d]
                            else:
                                tp = t - 1 if d == 0 else t + 1
                                rhs_prev = sb_hsT[:, d, :, BC * tp:BC * (tp + 1)]
                            ps = plp.tile([128, 8, BC], dt.float32, tag=f"psl{d}")
                            if gx_inject:
                                nc.tensor.matmul(
                                    ps[:], lhsT=sb_ident8[:],
                                    rhs=sb_gx[:, d, :, BC * t:BC * (t + 1)],
                                    start=True, stop=False)
                            for m in range(8):
                                for k in range(2):
                                    nc.tensor.matmul(
                                        ps[:, m, :],
                                        lhsT=sb_whh[:, d, k, m, :],
                                        rhs=rhs_prev[:, k, :],
                                        start=not gx_inject and m == 0 and k == 0,
                                        stop=(m == 7 and k == 1))
                            gsrc = ps
                            if not gx_inject:
                                gadd = pl.tile([128, 8, BC], dt.float32,
                                               tag=f"ga{d}")
                                nc.vector.tensor_add(
                                    gadd[:], ps[:],
                                    sb_gx[:, d, :, BC * t:BC * (t + 1)])
                                gsrc = gadd
                            if one_sig:
                                sig = pl.tile([128, 8, BC], dt.float32,
                                              tag=f"sig{d}")
                                nc.scalar.activation(sig[:], gsrc[:], AF.Sigmoid)
                                tg2 = pl.tile([128, 2, BC], dt.float32,
                                              tag=f"tg{d}")
                                nc.vector.tensor_scalar(
                                    tg2[:], sig[:, 6:8, :], 2.0, -1.0,
                                    mybir.AluOpType.mult, mybir.AluOpType.add)
                            else:
                                sig = pl.tile([128, 6, BC], dt.float32,
                                              tag=f"sig{d}")
                                nc.scalar.activation(sig[:], gsrc[:, 0:6, :],
                                                     AF.Sigmoid)
                                tg2 = pl.tile([128, 2, BC], dt.float32,
                                              tag=f"tg{d}")
                                nc.scalar.activation(tg2[:], gsrc[:, 6:8, :],
                                                     AF.Tanh)
                            t1 = pl.tile([128, 2, BC], dt.float32, tag=f"t1{d}")
                            t2 = pl.tile([128, 2, BC], dt.float32, tag=f"t2{d}")
                            nc.vector.tensor_mul(t1[:], sig[:, 2:4, :], sb_c[:, d])
                            nc.vector.tensor_mul(t2[:], sig[:, 0:2, :], tg2[:])
                            nc.vector.tensor_add(sb_c[:, d], t1[:], t2[:])
                            tch = pl.tile([128, 2, BC], dt.float32, tag=f"tc{d}")
                            nc.scalar.activation(tch[:], sb_c[:, d], AF.Tanh)
                            nc.vector.tensor_mul(
                                sb_hsT[:, d, :, BC * t:BC * (t + 1)],
                                sig[:, 4:6, :], tch[:])

                if stop_after == 'lstm':
                    continue
                # ---- phase D: feats -> EM / sel ----
                with tc.tile_pool(name="pfeat_ps", bufs=4, space="PSUM") as pfp:
                    for n0 in range(0, NT, NCH):
                        psf = pfp.tile([K, NCH], dt.float32, tag="psf")
                        for kk in range(4):
                            nc.tensor.matmul(
                                psf[:], lhsT=sb_wlin[:, kk, :],
                                rhs=sb_hsT[:, kk // 2, kk % 2, n0:n0 + NCH],
                                start=(kk == 0), stop=(kk == 3))
                        nc.scalar.activation(
                            sb_em[:, n0:n0 + NCH], psf[:], AF.Exp,
                            bias=sb_blin[:, 0:1])
                        nc.vector.tensor_mul(
                            sb_sel[:, n0:n0 + NCH], psf[:],
                            sb_oht[:, n0:n0 + NCH])

                if stop_after == 'feats':
                    continue
                # ---- phase E: CRF scan (independent batch chains) +
                # emission reduction ----
                for _cr in range(crf_reps):
                  with tc.tile_pool(name="pred", bufs=4) as pr, \
                     tc.tile_pool(name="pred_ps", bufs=2, space="PSUM") as prp:
                    sel_v = sb_sel[:].rearrange("j (t b) -> j b t", b=BC)
                    for b in range(BC):
                        nc.vector.tensor_reduce(
                            out=sb_emsum[:, b:b + 1], in_=sel_v[:, b, :],
                            axis=mybir.AxisListType.X, op=mybir.AluOpType.add)
                    pse = prp.tile([1, BC], dt.float32, tag="pse")
                    nc.tensor.matmul(pse[:], lhsT=sb_ones9[:], rhs=sb_emsum[:],
                                     start=True, stop=True)
                    nc.vector.tensor_copy(sb_res[0:1, 0:BC], pse[:])

                    nchain = max(1, min(crf_chains, BC))
                    w = BC // nchain
                    nc.vector.tensor_scalar_mul(
                        beta[:], sb_em[:, 0:BC], sb_estart[:, 0:1])
                    for t in range(1, t_steps):
                        for ci in range(nchain):
                            cs = slice(ci * w, (ci + 1) * w)
                            psb = prp.tile([K, w], dt.float32, tag=f"psb{ci}")
                            nc.tensor.matmul(
                                psb[:], lhsT=sb_et[:], rhs=beta[:, cs],
                                start=True, stop=True)
                            nc.vector.tensor_mul(
                                beta[:, cs], psb[:],
                                sb_em[:, BC * t + ci * w: BC * t + (ci + 1) * w])
                    bend = pr.tile([K, BC], dt.float32, tag="bend")
                    nc.vector.tensor_scalar_mul(bend[:], beta[:],
                                                sb_eend[:, 0:1])
                    psz = prp.tile([1, BC], dt.float32, tag="psz")
                    nc.tensor.matmul(psz[:], lhsT=sb_ones9[:], rhs=bend[:],
                                     start=True, stop=True)
                    lnz = pr.tile([1, BC], dt.float32, tag="lnz")
                    nc.scalar.activation(lnz[:], psz[:], AF.Ln)
                    nc.vector.tensor_scalar_add(
                        sb_res[0:1, BC:2 * BC], lnz[:],
                        float((t_steps - 1) * LOG_K))

            nc.sync.dma_start(out=d_res, in_=sb_res[:])

    nc.compile()
    return nc


def _prep_core_inputs(inputs, core, t_steps=T):
    """Host-side: slice batch shard + lay out tensors exactly as SBUF wants."""
    b0 = core * BC
    texts = np.asarray(inputs["texts"])[b0:b0 + BC, :t_steps]   # (BC, T)
    tags = np.asarray(inputs["tags"])[b0:b0 + BC, :t_steps]

    NT = t_steps * BC
    flat = texts.T.reshape(NT)                      # col c = t*BC + b
    tidx = flat.reshape(NT // 128, 128).T.astype(np.int32).copy()

    oht = np.zeros((K, NT), np.float32)
    tg_flat = tags.T.reshape(NT)
    oht[tg_flat, np.arange(NT)] = 1.0

    h0 = np.asarray(inputs["h0"])[:, b0:b0 + BC]    # (2, BC, 256)
    c0 = np.asarray(inputs["c0"])[:, b0:b0 + BC]
    h0q = np.ascontiguousarray(
        h0.reshape(2, BC, 2, 128).transpose(3, 0, 2, 1)).astype(BF16)
    c0i = np.ascontiguousarray(
        c0.reshape(2, BC, 2, 128).transpose(3, 0, 2, 1)).astype(np.float32)

    return {"tidx": tidx, "oht": oht, "h0q": h0q, "c0i": c0i}


def _prep_shared_inputs(inputs, one_sig=True):
    embed = np.asarray(inputs["embed"])
    embq = embed.astype(BF16)

    def lhsT_pack(W):
        """W (1024, 256) -> [p, khalf, m, q]; if one_sig, g-gate rows are
        scaled by 2 so a single sigmoid computes every gate
        (tanh(x) = 2 sigmoid(2x) - 1)."""
        out = np.zeros((128, 2, 8, 128), np.float32)
        for k in range(2):
            for mi, mo in enumerate(MORDER):
                blk = W[128 * mo:128 * (mo + 1), 128 * k:128 * (k + 1)]
                if one_sig and mi >= 6:
                    blk = blk * 2.0
                out[:, k, mi, :] = blk.T
        return out

    wih = np.stack([lhsT_pack(np.asarray(inputs["Wih_f"])),
                    lhsT_pack(np.asarray(inputs["Wih_r"]))], axis=1)
    whh = np.stack([lhsT_pack(np.asarray(inputs["Whh_f"])),
                    lhsT_pack(np.asarray(inputs["Whh_r"]))], axis=1)
    wih = np.ascontiguousarray(wih).astype(F8)
    whh = np.ascontiguousarray(whh).astype(F8)

    def bias_pack(bvec):
        out = np.stack([bvec[128 * mo:128 * (mo + 1)] for mo in MORDER])
        out = out.astype(np.float64)
        if one_sig:
            out[6:8] *= 2.0
        return out

    gbias = np.stack([bias_pack(np.asarray(inputs["b_f"])),
                      bias_pack(np.asarray(inputs["b_r"]))])
    gbias = np.ascontiguousarray(gbias.transpose(2, 0, 1)).astype(np.float32)

    W_lin = np.asarray(inputs["W_lin"])
    wlin = np.zeros((128, 4, K), np.float32)
    for kk in range(4):
        wlin[:, kk, :] = W_lin[:, 128 * kk:128 * (kk + 1)].T
    wlin = wlin.astype(F8)

    blin = np.asarray(inputs["b_lin"]).reshape(K, 1).astype(np.float32)
    trans = np.asarray(inputs["trans"]).astype(np.float64)
    et = np.exp(trans - LOG_K).astype(np.float32)
    estart = np.exp(np.asarray(inputs["start_trans"], np.float64)).reshape(K, 1).astype(np.float32)
    eend = np.exp(np.asarray(inputs["end_trans"], np.float64)).reshape(K, 1).astype(np.float32)

    return {"embq": embq, "wih": wih, "whh": whh, "gbias": gbias,
            "wlin": wlin, "blin": blin, "et": et, "estart": estart,
            "eend": eend}


def host_combine(inputs, res_list, t_steps=T):
    """res_list[c] = (1, 2*BC): [0,:BC] emission-sum, [0,BC:] logZ."""
    tags = np.asarray(inputs["tags"])[:, :t_steps]
    start = np.asarray(inputs["start_trans"], np.float64)
    end = np.asarray(inputs["end_trans"], np.float64)
    trans = np.asarray(inputs["trans"], np.float64)
    blin = np.asarray(inputs["b_lin"], np.float64)

    em_sums = np.concatenate([np.asarray(r, np.float64)[0, :BC] for r in res_list])
    logZ = np.concatenate([np.asarray(r, np.float64)[0, BC:] for r in res_list])

    tg = tags.T
    hostscore = (start[tg[0]] + trans[tg[:-1], tg[1:]].sum(0) + end[tg[-1]]
                 + blin[tg].sum(0))
    loss = -np.mean(em_sums + hostscore - logZ)
    return np.float32(loss)


def kernel(**inputs):
    from concourse.bass_utils import run_bass_kernel_spmd

    if "nc" not in _CACHE:
        _CACHE["nc"] = _build_module(T)
    nc = _CACHE["nc"]

    shared = _prep_shared_inputs(inputs)
    in_maps = []
    for c in range(NCORES):
        m = dict(shared)
        m.update(_prep_core_inputs(inputs, c))
        in_maps.append(m)

    out = run_bass_kernel_spmd(nc, in_maps, core_ids=list(range(NCORES)))
    res_list = [out.results[c]["res"] for c in range(NCORES)]
    return host_combine(inputs, res_list)



# revision 15
# speedup vs baseline: 1.2764x; 1.2764x over previous
"""BiLSTM-CRF loss kernel for Trainium2 (8 NeuronCores, SPMD data-parallel).

Full inputs -> full scalar output. Sharding: batch 32 -> 4 rows/core x 8 cores.

Per-core pipeline (v4):
  gather embeddings (indirect DMA) -> PE-transpose -> input projections Gx
  (fp8 weights) -> both LSTM directions step-interleaved -> linear projection
  + exp emissions -> chunked-parallel CRF scan -> per-batch partials.

LSTM step (v4): gates accumulation starts with an identity-matmul injecting
the precomputed Gx block, then 16 fp8 Whh tile matmuls.  One sigmoid covers
all four gates (g-gate rows pre-scaled by 2 on the host; tanh(x) =
2 sigmoid(2x) - 1).  The cell update uses two fused scalar_tensor_tensor ops:
  u = (sig_g - 0.5) * sig_i ;  c = 2*u + sig_f*c
so the recurrent chain is sigmoid -> t1/u -> c -> tanh -> hmul (4 DVE + 2 Act
ops per direction-step).

CRF (v4): the per-t recurrence beta_t = diag(em_t) ET'^T beta_{t-1} (with
ET' = exp(trans - log K)) is run as 28 chunk-operator chains (4 rows x 7
chunks of 73 steps), packed 14 chains per 126-partition block-diagonal
matmul.  The stationary operand (block-diag ET') never changes, so the scan
is 73 slots x 2 groups of [matmul -> per-partition scale] with the scale
alternating between the Act and DVE engines.  Chunk operators are then
transposed once on the PE and combined in 7 block-sparse matmuls; logZ =
ln(sum beta_T * exp(end)) + 511 log K.  Validated numerically: loss rel err
~4e-5 vs f64 reference.
"""

import numpy as np
import ml_dtypes

VOCAB, EMB, HID, K, B, T = 30000, 256, 512, 9, 32, 512
H = HID // 2          # 256 per-direction hidden
NCORES = 8
BC = B // NCORES      # 4 batch rows per core
LOG_K = float(np.log(K))
# m-chunk order in the gates psum tile: [i0 i1 f0 f1 o0 o1 g0 g1]
MORDER = [0, 1, 2, 3, 6, 7, 4, 5]

# CRF chunking: 511 transition steps = NCHUNK chunks of CLEN
NCHUNK, CLEN = 7, 73
NCHAIN = BC * NCHUNK          # 28 chains of 9 states
NGRP = 2                      # chains per matmul group = 14 (126 partitions)
CPG = NCHAIN // NGRP          # 14

F8 = ml_dtypes.float8_e4m3
BF16 = ml_dtypes.bfloat16

_CACHE = {}


def _build_module(t_steps=T):
    import concourse.bacc as bacc
    import concourse.tile as tile
    import concourse.mybir as mybir
    from concourse import bass
    from concourse.masks import make_identity

    dt = mybir.dt
    AF = mybir.ActivationFunctionType
    ALU = mybir.AluOpType
    NT = t_steps * BC  # flattened (t, b) columns per core

    nc = bacc.Bacc("TRN2", target_bir_lowering=False, debug=False,
                   num_devices=NCORES)

    d_emb = nc.dram_tensor("embq", [VOCAB, EMB], dt.bfloat16, kind="ExternalInput").ap()
    d_tidx = nc.dram_tensor("tidx", [128, NT // 128], dt.int32, kind="ExternalInput").ap()
    d_wih = nc.dram_tensor("wih", [128, 2, 2, 8, 128], dt.float8e4, kind="ExternalInput").ap()
    d_whh = nc.dram_tensor("whh", [128, 2, 2, 8, 128], dt.float8e4, kind="ExternalInput").ap()
    d_gbias = nc.dram_tensor("gbias", [128, 2, 8], dt.float32, kind="ExternalInput").ap()
    d_wlin = nc.dram_tensor("wlin", [128, 4, K], dt.float8e4, kind="ExternalInput").ap()
    d_blin = nc.dram_tensor("blin", [K, 1], dt.float32, kind="ExternalInput").ap()
    d_etbd = nc.dram_tensor("etbd", [128, 128], dt.bfloat16, kind="ExternalInput").ap()
    d_sinit = nc.dram_tensor("sinit", [128, 128], dt.bfloat16, kind="ExternalInput").ap()
    d_pmat = nc.dram_tensor("pmat", [K, CPG, 128], dt.bfloat16, kind="ExternalInput").ap()
    d_shift9 = nc.dram_tensor("shift9", [128, 128], dt.bfloat16, kind="ExternalInput").ap()
    d_gath = nc.dram_tensor("gath", [128, 2, K], dt.bfloat16, kind="ExternalInput").ap()
    d_estart = nc.dram_tensor("estart", [K, 1], dt.float32, kind="ExternalInput").ap()
    d_eend = nc.dram_tensor("eend", [K, 1], dt.float32, kind="ExternalInput").ap()
    d_oht = nc.dram_tensor("oht", [K, NT], dt.float32, kind="ExternalInput").ap()
    d_h0 = nc.dram_tensor("h0q", [128, 2, 2, BC], dt.bfloat16, kind="ExternalInput").ap()
    d_c0 = nc.dram_tensor("c0i", [128, 2, 2, BC], dt.float32, kind="ExternalInput").ap()
    d_res = nc.dram_tensor("res", [1, 2 * BC], dt.float32, kind="ExternalOutput").ap()

    with tile.TileContext(nc) as tc:
        from contextlib import ExitStack
        with ExitStack() as ctx:
            pconst = ctx.enter_context(tc.tile_pool(name="pconst", bufs=1))

            # ---- persistent SBUF tensors ----
            sb_wih = pconst.tile([128, 2, 2, 8, 128], dt.float8e4)
            sb_whh = pconst.tile([128, 2, 2, 8, 128], dt.float8e4)
            sb_gbias = pconst.tile([128, 2, 8], dt.float32)
            sb_wlin = pconst.tile([128, 4, K], dt.float8e4)
            sb_blin = pconst.tile([K, 1], dt.float32)
            sb_etbd = pconst.tile([128, 128], dt.bfloat16)
            sb_estart = pconst.tile([K, 1], dt.float32)
            sb_eend = pconst.tile([K, 1], dt.float32)
            sb_ones9 = pconst.tile([K, 1], dt.float32)
            sb_oht = pconst.tile([K, NT], dt.float32)
            sb_tidx = pconst.tile([128, NT // 128], dt.int32)
            sb_h0 = pconst.tile([128, 2, 2, BC], dt.bfloat16)
            sb_c = pconst.tile([128, 2, 2, BC], dt.float32)   # running c state
            sb_ident = pconst.tile([128, 128], dt.bfloat16)   # for PE transpose
            sb_ident8 = pconst.tile([128, 128], dt.float8e4)  # for Gx injection
            sb_xT = pconst.tile([128, 2, NT], dt.bfloat16)
            sb_gx = pconst.tile([128, 2, 8, NT], dt.bfloat16)
            sb_hsT = pconst.tile([128, 2, 2, NT], dt.bfloat16)  # [p, dir, khalf, col]
            sb_em = pconst.tile([K, NT], dt.float32)
            sb_sel = pconst.tile([K, NT], dt.float32)
            sb_emsum = pconst.tile([K, BC], dt.float32)
            sb_res = pconst.tile([1, 2 * BC], dt.float32)
            # CRF chunked-scan state
            sb_S = [pconst.tile([128, 128], dt.bfloat16, name=f"sb_S{g}")
                    for g in range(NGRP)]
            sb_STs = [pconst.tile([128, 128], dt.bfloat16, name=f"sb_STs{g}")
                      for g in range(NGRP)]
            sb_STp = [pconst.tile([128, 128], dt.bfloat16, name=f"sb_STp{g}")
                      for g in range(NGRP)]
            sb_emg = [pconst.tile([128, CLEN], dt.float32, name=f"sb_emg{g}")
                      for g in range(NGRP)]
            sb_brhs = [pconst.tile([128, 2], dt.bfloat16, name=f"sb_brhs{g}")
                       for g in range(NGRP)]
            sb_pmat = pconst.tile([K, CPG, 128], dt.bfloat16)
            sb_shift9 = pconst.tile([128, 128], dt.bfloat16)
            sb_gath = pconst.tile([128, 2, K], dt.bfloat16)
            sb_emq = pconst.tile([K, NT], dt.bfloat16)
            sb_beta0 = pconst.tile([K, BC], dt.float32)
            sb_beta0q = pconst.tile([K, BC], dt.bfloat16)
            sb_bend = pconst.tile([K, BC], dt.float32)

            eng = nc.sync  # DMA queue engine
            eng.dma_start(out=sb_wih[:], in_=d_wih)
            eng.dma_start(out=sb_whh[:], in_=d_whh)
            eng.dma_start(out=sb_gbias[:], in_=d_gbias)
            eng.dma_start(out=sb_wlin[:], in_=d_wlin)
            eng.dma_start(out=sb_blin[:], in_=d_blin)
            eng.dma_start(out=sb_etbd[:], in_=d_etbd)
            eng.dma_start(out=sb_S[0][:], in_=d_sinit)
            eng.dma_start(out=sb_S[1][:], in_=d_sinit)
            eng.dma_start(out=sb_estart[:], in_=d_estart)
            eng.dma_start(out=sb_eend[:], in_=d_eend)
            eng.dma_start(out=sb_oht[:], in_=d_oht)
            eng.dma_start(out=sb_tidx[:], in_=d_tidx)
            eng.dma_start(out=sb_h0[:], in_=d_h0)
            eng.dma_start(out=sb_c[:], in_=d_c0)
            eng.dma_start(out=sb_pmat[:], in_=d_pmat)
            eng.dma_start(out=sb_shift9[:], in_=d_shift9)
            eng.dma_start(out=sb_gath[:], in_=d_gath)
            nc.vector.memset(sb_ones9[:], 1.0)
            make_identity(nc, sb_ident[:])
            make_identity(nc, sb_ident8[:])

            NCH = min(512, NT)

            # ---- phase A: gather + transpose ----
            # order so that the chunks needed first by phase B/C come first
            iorder = list(range(0, 4)) + list(range(12, 16)) + list(range(4, 12))
            if NT // 128 != 16:
                iorder = list(range(NT // 128))
            with tc.tile_pool(name="pgather", bufs=4) as pg, \
                 tc.tile_pool(name="pg_ps", bufs=4, space="PSUM") as pgp:
                for i in iorder:
                    xg = pg.tile([128, EMB], dt.bfloat16, tag="xg")
                    nc.gpsimd.indirect_dma_start(
                        out=xg[:],
                        out_offset=None,
                        in_=d_emb,
                        in_offset=bass.IndirectOffsetOnAxis(
                            ap=sb_tidx[:, i:i + 1], axis=0),
                    )
                    for k in range(2):
                        pst = pgp.tile([128, 128], dt.bfloat16, tag="pst")
                        nc.tensor.transpose(
                            out=pst[:], in_=xg[:, 128 * k:128 * (k + 1)],
                            identity=sb_ident[:])
                        nc.scalar.copy(
                            out=sb_xT[:, k, 128 * i:128 * (i + 1)],
                            in_=pst[:])

            # ---- phase B: input projections for both directions ----
            # interleave (dir, chunk) so dir-0's first chunk and dir-1's last
            # chunk are produced first (phase C consumes them first)
            nchk = NT // NCH
            chunk_order = []
            for j in range(nchk):
                chunk_order.append((0, j * NCH))
                chunk_order.append((1, (nchk - 1 - j) * NCH))
            with tc.tile_pool(name="pproj", bufs=4, space="PSUM") as ppp:
                for ci, (d, n0) in enumerate(chunk_order):
                    for m in range(8):
                        psp = ppp.tile([128, NCH], dt.float32, tag="psp")
                        for k in range(2):
                            nc.tensor.matmul(
                                psp[:], lhsT=sb_wih[:, d, k, m, :],
                                rhs=sb_xT[:, k, n0:n0 + NCH],
                                start=(k == 0), stop=(k == 1))
                        if m % 2 == 0:
                            nc.scalar.activation(
                                sb_gx[:, d, m, n0:n0 + NCH], psp[:],
                                AF.Identity, bias=sb_gbias[:, d, m:m + 1])
                        else:
                            nc.vector.tensor_scalar_add(
                                sb_gx[:, d, m, n0:n0 + NCH], psp[:],
                                sb_gbias[:, d, m:m + 1])

            # ---- phase C: both LSTM recurrences, step-interleaved ----
            with tc.tile_pool(name="plstm", bufs=4) as pl, \
                 tc.tile_pool(name="plstm_ps", bufs=4, space="PSUM") as plp:
                for s in range(t_steps):
                    for d in range(2):
                        t = s if d == 0 else t_steps - 1 - s
                        if s == 0:
                            rhs_prev = sb_h0[:, d]
                        else:
                            tp = t - 1 if d == 0 else t + 1
                            rhs_prev = sb_hsT[:, d, :, BC * tp:BC * (tp + 1)]
                        ps = plp.tile([128, 8, BC], dt.float32, tag=f"psl{d}")
                        nc.tensor.matmul(
                            ps[:], lhsT=sb_ident8[:],
                            rhs=sb_gx[:, d, :, BC * t:BC * (t + 1)],
                            start=True, stop=False)
                        for m in range(8):
                            for k in range(2):
                                nc.tensor.matmul(
                                    ps[:, m, :],
                                    lhsT=sb_whh[:, d, k, m, :],
                                    rhs=rhs_prev[:, k, :],
                                    start=False,
                                    stop=(m == 7 and k == 1))
                        # sig chunks: [0:2]=i, [2:4]=f, [4:6]=o, [6:8]=sig(2g)
                        sig = pl.tile([128, 8, BC], dt.float32, tag=f"sig{d}")
                        nc.scalar.activation(sig[:], ps[:], AF.Sigmoid)
                        t1 = pl.tile([128, 2, BC], dt.float32, tag=f"t1{d}")
                        nc.vector.tensor_mul(t1[:], sig[:, 2:4, :], sb_c[:, d])
                        u = pl.tile([128, 2, BC], dt.float32, tag=f"u{d}")
                        nc.vector.scalar_tensor_tensor(
                            u[:], sig[:, 6:8, :], 0.5, sig[:, 0:2, :],
                            ALU.subtract, ALU.mult)
                        nc.vector.scalar_tensor_tensor(
                            sb_c[:, d], u[:], 2.0, t1[:], ALU.mult, ALU.add)
                        tch = pl.tile([128, 2, BC], dt.float32, tag=f"tc{d}")
                        nc.scalar.activation(tch[:], sb_c[:, d], AF.Tanh)
                        nc.vector.tensor_mul(
                            sb_hsT[:, d, :, BC * t:BC * (t + 1)],
                            sig[:, 4:6, :], tch[:])

            # ---- phase D: feats -> EM / sel ----
            with tc.tile_pool(name="pfeat_ps", bufs=4, space="PSUM") as pfp:
                for n0 in range(0, NT, NCH):
                    psf = pfp.tile([K, NCH], dt.float32, tag="psf")
                    for kk in range(4):
                        nc.tensor.matmul(
                            psf[:], lhsT=sb_wlin[:, kk, :],
                            rhs=sb_hsT[:, kk // 2, kk % 2, n0:n0 + NCH],
                            start=(kk == 0), stop=(kk == 3))
                    nc.scalar.activation(
                        sb_em[:, n0:n0 + NCH], psf[:], AF.Exp,
                        bias=sb_blin[:, 0:1])
                    nc.vector.tensor_mul(
                        sb_sel[:, n0:n0 + NCH], psf[:],
                        sb_oht[:, n0:n0 + NCH])

            # ---- phase E: emission reduction + chunked CRF scan ----
            with tc.tile_pool(name="pscan_ps", bufs=2, space="PSUM") as prp:
                # gold-path emission sums
                sel_v = sb_sel[:].rearrange("j (t b) -> j b t", b=BC)
                for b in range(BC):
                    nc.vector.tensor_reduce(
                        out=sb_emsum[:, b:b + 1], in_=sel_v[:, b, :],
                        axis=mybir.AxisListType.X, op=mybir.AluOpType.add)
                pse = prp.tile([1, BC], dt.float32, tag="pse")
                nc.tensor.matmul(pse[:], lhsT=sb_ones9[:], rhs=sb_emsum[:],
                                 start=True, stop=True)
                nc.vector.tensor_copy(sb_res[0:1, 0:BC], pse[:])

                # beta0 = estart * em[:, t=0]
                nc.vector.tensor_scalar_mul(
                    sb_beta0[:], sb_em[:, 0:BC], sb_estart[:, 0:1])
                nc.vector.tensor_copy(sb_beta0q[:], sb_beta0[:])

                # bf16 copy of em for the placement matmuls
                nc.vector.tensor_copy(sb_emq[:, 0:NT // 2], sb_em[:, 0:NT // 2])
                nc.scalar.copy(sb_emq[:, NT // 2:NT], sb_em[:, NT // 2:NT])

                # stage emissions per chain via placement matmuls: chain
                # X = g*CPG + j covers row b = X//NCHUNK, chunk q = X%NCHUNK;
                # em_g[9j+r, s] = em[r, (1 + q*CLEN + s)*BC + b]
                emq_v = sb_emq[:].rearrange("j (t b) -> j b t", b=BC)
                for g in range(NGRP):
                    psE = prp.tile([128, CLEN], dt.float32, tag="psE")
                    for j in range(CPG):
                        X = g * CPG + j
                        b, q = X // NCHUNK, X % NCHUNK
                        nc.tensor.matmul(
                            psE[:], lhsT=sb_pmat[:, j, :],
                            rhs=emq_v[:, b, 1 + q * CLEN: 1 + (q + 1) * CLEN],
                            start=(j == 0), stop=(j == CPG - 1))
                    if g == 0:
                        nc.scalar.copy(sb_emg[g][:], psE[:])
                    else:
                        nc.vector.tensor_copy(sb_emg[g][:], psE[:])

                # chunk-operator scan: S_g <- diag(em_s) @ ET'^T @ S_g
                for s in range(CLEN):
                    for g in range(NGRP):
                        psS = prp.tile([128, 126], dt.float32, tag=f"psS{g}")
                        nc.tensor.matmul(
                            psS[:], lhsT=sb_etbd[:, 0:128],
                            rhs=sb_S[g][:, 0:126], start=True, stop=True)
                        if g == 0:
                            nc.scalar.mul(sb_S[g][:, 0:126], psS[:],
                                          sb_emg[g][:, s:s + 1])
                        else:
                            nc.vector.tensor_scalar_mul(
                                sb_S[g][:, 0:126], psS[:],
                                sb_emg[g][:, s:s + 1])

            with tc.tile_pool(name="pred", bufs=4) as pr, \
                 tc.tile_pool(name="pred_ps", bufs=1, space="PSUM") as prp:
                # transpose chunk operators; the shifted variant bakes in the
                # +9-partition block advance used by the combine walk
                for g in range(NGRP):
                    psT = prp.tile([128, 128], dt.bfloat16, tag="psT")
                    nc.tensor.transpose(out=psT[:], in_=sb_S[g][:],
                                        identity=sb_shift9[:])
                    nc.vector.tensor_copy(sb_STs[g][:], psT[:])
                    psT2 = prp.tile([128, 128], dt.bfloat16, tag="psT2")
                    nc.tensor.transpose(out=psT2[:], in_=sb_S[g][:],
                                        identity=sb_ident[:])
                    nc.vector.tensor_copy(sb_STp[g][:], psT2[:])

                # init combine rhs via placement matmuls: group g cols are
                # rows (2g, 2g+1) at blocks (0, NCHUNK)
                for g in range(NGRP):
                    z0 = pr.tile([K, 2], dt.bfloat16, tag="z0")
                    z7 = pr.tile([K, 2], dt.bfloat16, tag="z7")
                    nc.vector.memset(z0[:], 0.0)
                    nc.vector.memset(z7[:], 0.0)
                    nc.vector.tensor_copy(z0[:, 0:1], sb_beta0q[:, 2 * g:2 * g + 1])
                    nc.vector.tensor_copy(z7[:, 1:2], sb_beta0q[:, 2 * g + 1:2 * g + 2])
                    psI = prp.tile([128, 2], dt.float32, tag="psI")
                    nc.tensor.matmul(psI[:], lhsT=sb_pmat[:, 0, :], rhs=z0[:],
                                     start=True, stop=False)
                    nc.tensor.matmul(psI[:], lhsT=sb_pmat[:, NCHUNK, :], rhs=z7[:],
                                     start=False, stop=True)
                    nc.vector.tensor_copy(sb_brhs[g][:], psI[:])

                # combine walk: q ascending; blocks advance (q, 7+q) ->
                # (q+1, 8+q) via the pre-shifted operator, except the last
                # step which uses the plain operator
                for q in range(NCHUNK):
                    for g in range(NGRP):
                        psb = prp.tile([128, 2], dt.float32, tag=f"psb{g}")
                        lhsT = sb_STs[g] if q + 1 < NCHUNK else sb_STp[g]
                        nc.tensor.matmul(psb[:], lhsT=lhsT[:], rhs=sb_brhs[g][:],
                                         start=True, stop=True)
                        nc.vector.tensor_copy(sb_brhs[g][:], psb[:])

                # extract final beta blocks (NCHUNK-1, CPG-1) -> [9, 2]
                for g in range(NGRP):
                    psF = prp.tile([K, 2], dt.float32, tag="psF")
                    nc.tensor.matmul(psF[:], lhsT=sb_gath[:, 0, :],
                                     rhs=sb_brhs[g][:], start=True, stop=False)
                    nc.tensor.matmul(psF[:], lhsT=sb_gath[:, 1, :],
                                     rhs=sb_brhs[g][:], start=False, stop=True)
                    nc.vector.tensor_copy(sb_bend[:, 2 * g:2 * g + 2], psF[:])

                # logZ = ln(sum beta_T * exp(end)) + 511 log K
                bend = pr.tile([K, BC], dt.float32, tag="bendx")
                nc.vector.tensor_scalar_mul(bend[:], sb_bend[:],
                                            sb_eend[:, 0:1])
                psz = prp.tile([1, BC], dt.float32, tag="psz")
                nc.tensor.matmul(psz[:], lhsT=sb_ones9[:], rhs=bend[:],
                                 start=True, stop=True)
                lnz = pr.tile([1, BC], dt.float32, tag="lnz")
                nc.scalar.activation(lnz[:], psz[:], AF.Ln)
                nc.vector.tensor_scalar_add(
                    sb_res[0:1, BC:2 * BC], lnz[:],
                    float((t_steps - 1) * LOG_K))

            nc.sync.dma_start(out=d_res, in_=sb_res[:])

    nc.compile()
    return nc


def _prep_core_inputs(inputs, core, t_steps=T):
    """Host-side: slice batch shard + lay out tensors exactly as SBUF wants."""
    b0 = core * BC
    texts = np.asarray(inputs["texts"])[b0:b0 + BC, :t_steps]   # (BC, T)
    tags = np.asarray(inputs["tags"])[b0:b0 + BC, :t_steps]

    NT = t_steps * BC
    flat = texts.T.reshape(NT)                      # col c = t*BC + b
    tidx = flat.reshape(NT // 128, 128).T.astype(np.int32).copy()

    oht = np.zeros((K, NT), np.float32)
    tg_flat = tags.T.reshape(NT)
    oht[tg_flat, np.arange(NT)] = 1.0

    h0 = np.asarray(inputs["h0"])[:, b0:b0 + BC]    # (2, BC, 256)
    c0 = np.asarray(inputs["c0"])[:, b0:b0 + BC]
    h0q = np.ascontiguousarray(
        h0.reshape(2, BC, 2, 128).transpose(3, 0, 2, 1)).astype(BF16)
    c0i = np.ascontiguousarray(
        c0.reshape(2, BC, 2, 128).transpose(3, 0, 2, 1)).astype(np.float32)

    return {"tidx": tidx, "oht": oht, "h0q": h0q, "c0i": c0i}


def _prep_shared_inputs(inputs, one_sig=True):
    embed = np.asarray(inputs["embed"])
    embq = embed.astype(BF16)

    def lhsT_pack(W):
        """W (1024, 256) -> [p, khalf, m, q]; g-gate rows are scaled by 2 so
        a single sigmoid computes every gate (tanh(x) = 2 sigmoid(2x) - 1)."""
        out = np.zeros((128, 2, 8, 128), np.float32)
        for k in range(2):
            for mi, mo in enumerate(MORDER):
                blk = W[128 * mo:128 * (mo + 1), 128 * k:128 * (k + 1)]
                if one_sig and mi >= 6:
                    blk = blk * 2.0
                out[:, k, mi, :] = blk.T
        return out

    wih = np.stack([lhsT_pack(np.asarray(inputs["Wih_f"])),
                    lhsT_pack(np.asarray(inputs["Wih_r"]))], axis=1)
    whh = np.stack([lhsT_pack(np.asarray(inputs["Whh_f"])),
                    lhsT_pack(np.asarray(inputs["Whh_r"]))], axis=1)
    wih = np.ascontiguousarray(wih).astype(F8)
    whh = np.ascontiguousarray(whh).astype(F8)

    def bias_pack(bvec):
        out = np.stack([bvec[128 * mo:128 * (mo + 1)] for mo in MORDER])
        out = out.astype(np.float64)
        if one_sig:
            out[6:8] *= 2.0
        return out

    gbias = np.stack([bias_pack(np.asarray(inputs["b_f"])),
                      bias_pack(np.asarray(inputs["b_r"]))])
    gbias = np.ascontiguousarray(gbias.transpose(2, 0, 1)).astype(np.float32)

    W_lin = np.asarray(inputs["W_lin"])
    wlin = np.zeros((128, 4, K), np.float32)
    for kk in range(4):
        wlin[:, kk, :] = W_lin[:, 128 * kk:128 * (kk + 1)].T
    wlin = wlin.astype(F8)

    blin = np.asarray(inputs["b_lin"]).reshape(K, 1).astype(np.float32)
    trans = np.asarray(inputs["trans"]).astype(np.float64)
    et = np.exp(trans - LOG_K).astype(np.float32)
    # block-diagonal ET' for the chunked CRF scan: 14 blocks of (K, K)
    etbd = np.zeros((128, 128), np.float32)
    for j in range(CPG):
        etbd[9 * j:9 * j + 9, 9 * j:9 * j + 9] = et
    etbd = etbd.astype(BF16)
    sinit = np.zeros((128, 128), np.float32)
    sinit[np.arange(126), np.arange(126)] = 1.0
    sinit = sinit.astype(BF16)
    # placement matrices: pmat[r, j, 9j+r] = 1 scatters a [9] block into
    # partition block j (via out = pmat[:, j, :].T @ rhs)
    pmat = np.zeros((K, CPG, 128), np.float32)
    for j in range(CPG):
        pmat[np.arange(K), j, 9 * j + np.arange(K)] = 1.0
    pmat = pmat.astype(BF16)
    # shift9[p, p+9] = 1: transpose(S, shift9) = S^T with free dim shifted +9
    shift9 = np.zeros((128, 128), np.float32)
    shift9[np.arange(119), 9 + np.arange(119)] = 1.0
    shift9 = shift9.astype(BF16)
    # gather matrices: block NCHUNK-1 (col 0) and CPG-1 (col 1) -> [9]
    gath = np.zeros((128, 2, K), np.float32)
    gath[9 * (NCHUNK - 1) + np.arange(K), 0, np.arange(K)] = 1.0
    gath[9 * (CPG - 1) + np.arange(K), 1, np.arange(K)] = 1.0
    gath = gath.astype(BF16)
    estart = np.exp(np.asarray(inputs["start_trans"], np.float64)).reshape(K, 1).astype(np.float32)
    eend = np.exp(np.asarray(inputs["end_trans"], np.float64)).reshape(K, 1).astype(np.float32)

    return {"embq": embq, "wih": wih, "whh": whh, "gbias": gbias,
            "wlin": wlin, "blin": blin, "etbd": etbd, "sinit": sinit,
            "pmat": pmat, "shift9": shift9, "gath": gath,
            "estart": estart, "eend": eend}


def host_combine(inputs, res_list, t_steps=T):
    """res_list[c] = (1, 2*BC): [0,:BC] emission-sum, [0,BC:] logZ."""
    tags = np.asarray(inputs["tags"])[:, :t_steps]
    start = np.asarray(inputs["start_trans"], np.float64)
    end = np.asarray(inputs["end_trans"], np.float64)
    trans = np.asarray(inputs["trans"], np.float64)
    blin = np.asarray(inputs["b_lin"], np.float64)

    em_sums = np.concatenate([np.asarray(r, np.float64)[0, :BC] for r in res_list])
    logZ = np.concatenate([np.asarray(r, np.float64)[0, BC:] for r in res_list])

    tg = tags.T
    hostscore = (start[tg[0]] + trans[tg[:-1], tg[1:]].sum(0) + end[tg[-1]]
                 + blin[tg].sum(0))
    loss = -np.mean(em_sums + hostscore - logZ)
    return np.float32(loss)


def kernel(**inputs):
    from concourse.bass_utils import run_bass_kernel_spmd

    if "nc" not in _CACHE:
        _CACHE["nc"] = _build_module(T)
    nc = _CACHE["nc"]

    shared = _prep_shared_inputs(inputs)
    in_maps = []
    for c in range(NCORES):
        m = dict(shared)
        m.update(_prep_core_inputs(inputs, c))
        in_maps.append(m)

    out = run_bass_kernel_spmd(nc, in_maps, core_ids=list(range(NCORES)))
    res_list = [out.results[c]["res"] for c in range(NCORES)]
    return host_combine(inputs, res_list)
